# revision 7
# baseline (speedup 1.0000x reference)
"""Trainium2 Bass kernel for batched greedy nearest-neighbor selection.

Algorithm (per batch): repeatedly take j* = argmin over unvisited columns of
distance[point], mark j* visited, move point to j*.  B=64 batches are sharded
8 per core across 8 NeuronCores (pure data parallel).  Each core runs its 8
chains in lockstep: one indirect-DMA gather fetches the 8 current rows, and a
wide [80 x 100] layout (partition p = 10*b + q holds row elements
j = q*100 + c) keeps every vector op ~100 elements deep.

Per step:
  R[80,100]   <- gather rows at OFF (indirect DMA)
  S = MNEG - R              (MNEG in {0, -BIG}: visited cells sink to -BIG-d)
  V/CI        <- per-partition top8 value/index of S  (InstMax / InstMaxIndex)
  VC          <- spread V,CI onto per-chunk columns; one PE matmul collapses
                 the 10 chunk-partitions of each batch into PSUM rows [8, 20]
  Vm/QI       <- per-batch argmax chunk q* (InstMax / InstMaxIndex, ties -> lowest q)
  c*          <- CI of winning chunk (masked reduce); j* = 100*q* + c*
  OFF         <- next-row DMA offsets from j*;  MNEG[j*] = -BIG (mask update)
  PREDT[:,t]  <- j*

Ties resolve to the lowest j exactly like argmin (InstMaxIndex returns the
first occurrence).  Steps only run to T = max(pred_len); the tail is padding,
applied host-side along with the active mask.
"""

import numpy as np

import concourse.bacc as bacc
import concourse.bass as bass
import concourse.mybir as mybir
import concourse.tile as tile
from concourse.bass_utils import run_bass_kernel_spmd

BIG = 1.0e6
N = 1000          # nodes per batch
NCORES = 8
BL = 8            # batches per core
Q, C = 10, 100    # row split: Q chunk-partitions x C columns (Q*C == N)
P = BL * Q        # 80 partitions in the wide layout

F32 = mybir.dt.float32
I32 = mybir.dt.int32
U32 = mybir.dt.uint32


def _const_data():
    p = np.arange(P)
    return {
        # [80,10] 1 where column q equals this partition's chunk index
        "qmask": (p[:, None] % Q == np.arange(Q)[None, :]).astype(np.float32),
        # [80,8] lhsT collapsing chunk partitions into batch rows (mm1)
        "emat": (p[:, None] // Q == np.arange(BL)[None, :]).astype(np.float32),
        # [8,80] lhsT broadcasting per-batch scalars back to 80 partitions (mm2)
        "selt": (np.arange(BL)[:, None] == p[None, :] // Q).astype(np.float32),
        # [80,100] global column index j carried by each cell
        "iotag": ((p[:, None] % Q) * C + np.arange(C)[None, :]).astype(np.float32),
        # [80,1] row-offset base: b*N*Q + q  (row of dist = (b*N + j)*Q ... + q)
        "qb80": ((p // Q) * (N * Q) + (p % Q)).astype(np.float32)[:, None],
        # [8,10] chunk indices per batch row
        "iota10": np.tile(np.arange(Q, dtype=np.float32), (BL, 1)),
    }


def build_program(T):
    nc = bacc.Bacc("TRN2", target_bir_lowering=False, debug=False)
    dist = nc.dram_tensor("dist", [BL * N * Q, C], F32, kind="ExternalInput").ap()
    mneg0 = nc.dram_tensor("mneg0", [P, C], F32, kind="ExternalInput").ap()
    off0 = nc.dram_tensor("off0", [P, 1], I32, kind="ExternalInput").ap()
    predo = nc.dram_tensor("pred", [BL, N], F32, kind="ExternalOutput").ap()

    cd = _const_data()
    cdram = {k: nc.inline_tensor(v, k).ap() for k, v in cd.items()}

    with tile.TileContext(nc) as tc:
        with (
            tc.tile_pool(name="const", bufs=1) as cpool,
            tc.tile_pool(name="state", bufs=1) as spool,
            tc.tile_pool(name="work", bufs=2) as wpool,
            tc.tile_pool(name="psum", bufs=2, space="PSUM") as ppool,
        ):
            QMASK = cpool.tile_from(cdram["qmask"])
            EMAT = cpool.tile_from(cdram["emat"])
            SELT = cpool.tile_from(cdram["selt"])
            IOTAG = cpool.tile_from(cdram["iotag"])
            QB80 = cpool.tile_from(cdram["qb80"])
            IOTA10 = cpool.tile_from(cdram["iota10"])
            NEGBIG = cpool.tile([P, C], F32)
            nc.vector.memset(NEGBIG[:], -BIG)

            MNEG = spool.tile_from(mneg0)
            OFF = spool.tile_from(off0)
            PREDT = spool.tile([BL, N], F32)

            for t in range(T):
                R = wpool.tile([P, C], F32, tag="R")
                nc.gpsimd.indirect_dma_start(
                    out=R[:],
                    out_offset=None,
                    in_=dist,
                    in_offset=bass.IndirectOffsetOnAxis(ap=OFF[:, :1], axis=0),
                )
                S = wpool.tile([P, C], F32, tag="S")
                nc.vector.tensor_tensor(
                    out=S[:], in0=MNEG[:], in1=R[:], op=mybir.AluOpType.subtract
                )
                V = wpool.tile([P, 8], F32, tag="V")
                CI = wpool.tile([P, 8], U32, tag="CI")
                nc.vector.max(out=V[:], in_=S[:])
                nc.vector.max_index(out=CI[:], in_max=V[:], in_values=S[:])
                CF = wpool.tile([P, 1], F32, tag="CF")
                nc.vector.tensor_copy(out=CF[:], in_=CI[:, 0:1])
                VC = wpool.tile([P, 2 * Q], F32, tag="VC")
                nc.vector.tensor_scalar(
                    out=VC[:, 0:Q], in0=QMASK[:], scalar1=V[:, 0:1],
                    scalar2=None, op0=mybir.AluOpType.mult,
                )
                nc.vector.tensor_scalar(
                    out=VC[:, Q:2 * Q], in0=QMASK[:], scalar1=CF[:],
                    scalar2=None, op0=mybir.AluOpType.mult,
                )
                WPS = ppool.tile([BL, 2 * Q], F32, tag="WPS")
                nc.tensor.matmul(out=WPS[:], lhsT=EMAT[:], rhs=VC[:], start=True, stop=True)
                WSB = wpool.tile([BL, 2 * Q], F32, tag="WSB")
                nc.vector.tensor_copy(out=WSB[:], in_=WPS[:])

                VM = wpool.tile([BL, 8], F32, tag="VM")
                QI = wpool.tile([BL, 8], U32, tag="QI")
                nc.vector.max(out=VM[:], in_=WSB[:, 0:Q])
                nc.vector.max_index(out=QI[:], in_max=VM[:], in_values=WSB[:, 0:Q])
                QF = wpool.tile([BL, 1], F32, tag="QF")
                nc.vector.tensor_copy(out=QF[:], in_=QI[:, 0:1])
                EQQ = wpool.tile([BL, Q], F32, tag="EQQ")
                nc.vector.tensor_scalar(
                    out=EQQ[:], in0=IOTA10[:], scalar1=QF[:],
                    scalar2=None, op0=mybir.AluOpType.is_equal,
                )
                TJ = wpool.tile([BL, Q], F32, tag="TJ")
                CSTAR = wpool.tile([BL, 1], F32, tag="CSTAR")
                nc.vector.tensor_tensor(out=TJ[:], in0=EQQ[:], in1=WSB[:, Q:2 * Q],
                                        op=mybir.AluOpType.mult)
                nc.vector.tensor_reduce(out=CSTAR[:], in_=TJ[:],
                                        axis=mybir.AxisListType.X, op=mybir.AluOpType.add)
                JF = wpool.tile([BL, 1], F32, tag="JF")
                nc.vector.tensor_scalar(
                    out=JF[:], in0=QF[:], scalar1=float(C), scalar2=CSTAR[:],
                    op0=mybir.AluOpType.mult, op1=mybir.AluOpType.add,
                )
                nc.vector.tensor_copy(out=PREDT[:, t:t + 1], in_=JF[:])

                JBCP = ppool.tile([P, 1], F32, tag="JBCP")
                nc.tensor.matmul(out=JBCP[:], lhsT=SELT[:], rhs=JF[:], start=True, stop=True)
                JBC = wpool.tile([P, 1], F32, tag="JBC")
                nc.vector.tensor_copy(out=JBC[:], in_=JBCP[:])
                nc.vector.tensor_scalar(
                    out=OFF[:], in0=JBC[:], scalar1=float(Q), scalar2=QB80[:],
                    op0=mybir.AluOpType.mult, op1=mybir.AluOpType.add,
                )
                EQ2 = wpool.tile([P, C], mybir.dt.uint8, tag="EQ2")
                nc.vector.tensor_scalar(
                    out=EQ2[:], in0=IOTAG[:], scalar1=JBC[:],
                    scalar2=None, op0=mybir.AluOpType.is_equal,
                )
                nc.vector.copy_predicated(out=MNEG[:], mask=EQ2[:], data=NEGBIG[:])

            nc.sync.dma_start(out=predo[:, :T], in_=PREDT[:, :T])
    nc.compile()
    return nc


def make_in_maps(distance, mask, start):
    """Shard full inputs into per-core in_maps for run_bass_kernel_spmd."""
    B = mask.shape[0]
    mneg_all = np.where(mask, np.float32(-BIG), np.float32(0.0)).astype(np.float32)
    mneg_all = mneg_all.reshape(B, Q, C)
    b = np.arange(BL)
    in_maps = []
    for core in range(NCORES):
        sl = slice(core * BL, (core + 1) * BL)
        dist_l = np.ascontiguousarray(distance[sl].reshape(BL * N * Q, C))
        mneg_l = np.ascontiguousarray(mneg_all[sl].reshape(P, C))
        off_l = (
            (b[:, None] * N + np.asarray(start[sl])[:, None]) * Q
            + np.arange(Q)[None, :]
        ).astype(np.int32).reshape(P, 1)
        in_maps.append({"dist": dist_l, "mneg0": mneg_l, "off0": off_l})
    return in_maps


_prog_cache = {}


def _get_program(T):
    if T not in _prog_cache:
        _prog_cache[T] = build_program(T)
    return _prog_cache[T]


def kernel(distance, mask, start_idx, pad_value):
    distance = np.ascontiguousarray(np.asarray(distance, dtype=np.float32))
    mask = np.asarray(mask).astype(bool)
    start = np.asarray(start_idx).astype(np.int64)
    pad = int(pad_value)
    B = mask.shape[0]

    pred_len = (N - mask.sum(axis=1)).astype(np.int32)
    T = max(int(pred_len.max()), 1)

    nc = _get_program(T)
    in_maps = make_in_maps(distance, mask, start)
    res = run_bass_kernel_spmd(nc, in_maps, core_ids=list(range(NCORES)))

    idx_all = np.concatenate(
        [np.rint(res.results[c]["pred"][:, :T]).astype(np.int32) for c in range(NCORES)],
        axis=0,
    )
    pred = np.full((B, N), pad, dtype=np.int32)
    active = np.arange(T)[None, :] < pred_len[:, None]
    pred[:, :T] = np.where(active, idx_all, pad)
    return pred, pred_len
